# revision 4
# baseline (speedup 1.0000x reference)
"""PositionalSparseLinear v4: pair-pooled dedup gather via indirect DMA +
compressed scatter-matrix PE accumulation.

Same algorithm as v3 but the pool gather uses one indirect_dma_start per
128-row chunk (idx [128,1] int32), since custom-ucode dma_gather does not
compile in this environment. Each gather group (pair; pair 0 split in two
halves) owns a dedicated semaphore and consumers wait only for a group's
full count, so unordered DMA completions cannot satisfy a wait early.
"""

import sys

sys.path.insert(0, "/opt/trn_rl_repo")

import numpy as np

from contextlib import ExitStack

import concourse.bass as bass
import concourse.mybir as mybir
from concourse.bass_utils import run_bass_kernel_spmd

B = 1024
IN = 8192
O = 8192
K = 32
NCORES = 8
OC = O // NCORES       # 1024
NT = OC // 128         # 8 tiles/core
NP = NT // 2           # 4 pairs/core
NBH = B // 512

F16 = mybir.dt.float16
F32 = mybir.dt.float32
I32 = mybir.dt.int32

_cached = {}


def _build_program(cmax):
    chalf = (cmax + 1) // 2
    nc = bass.Bass()
    xT_in = nc.declare_dram_parameter("xT16", [IN, B], F16, isOutput=False)
    st_in = nc.declare_dram_parameter("stat", [NT, 128, cmax * 128], F16, isOutput=False)
    gi_in = nc.declare_dram_parameter("gidx", [128, NP, cmax], I32, isOutput=False)
    y_out = nc.declare_dram_parameter("y", [NT, 128, B], F32, isOutput=True)

    with (
        nc.sbuf_tensor("pool_sb", [128, 2, cmax, B], F16) as pool_sb,
        nc.sbuf_tensor("st_sb", [128, 2, cmax * 128], F16) as st_sb,
        nc.sbuf_tensor("gi_sb", [128, NP, cmax], I32) as gi_sb,
        nc.sbuf_tensor("out_sb", [128, 2, B], F32) as out_sb,
        ExitStack() as _stack,
        nc.Block() as block,
        nc.semaphore("i_sem") as i_sem,
        nc.semaphore("gh0a") as gh0a,        # pair 0 first half
        nc.semaphore("gh0b") as gh0b,        # pair 0 second half
        nc.semaphore("g1") as g1,
        nc.semaphore("g2") as g2,
        nc.semaphore("g3") as g3,
        nc.semaphore("st_sem0") as st_sem0,
        nc.semaphore("st_sem1") as st_sem1,
        nc.semaphore("pe_sem") as pe_sem,
        nc.semaphore("v_sem") as v_sem,
        nc.semaphore("yd_sem0") as yd_sem0,
        nc.semaphore("yd_sem1") as yd_sem1,
    ):
        _ps = [
            _stack.enter_context(nc.psum_tensor(f"ps{i}", [128, 512], F32))
            for i in range(8)
        ]
        psum = [(_ps[0], _ps[1]), (_ps[2], _ps[3]), (_ps[4], _ps[5]), (_ps[6], _ps[7])]
        st_sems = [st_sem0, st_sem1]
        yd_sems = [yd_sem0, yd_sem1]
        pair_sems = [None, g1, g2, g3]

        @block.sync
        def _(sync: bass.BassEngine):
            sync.dma_start(out=gi_sb[:], in_=gi_in[:]).then_inc(i_sem, 16)
            for T in range(NT):
                if T >= 2:
                    sync.wait_ge(pe_sem, T - 1)
                sync.dma_start(out=st_sb[:, T % 2], in_=st_in[T]).then_inc(
                    st_sems[T % 2], 16
                )

        @block.gpsimd
        def _(gpsimd: bass.BassGpSimd):
            gpsimd.wait_ge(i_sem, 16)
            for p in range(NP):
                if p >= 2:
                    gpsimd.wait_ge(pe_sem, 2 * p - 2)
                for cn in range(cmax):
                    if p == 0:
                        sem = gh0a if cn < chalf else gh0b
                    else:
                        sem = pair_sems[p]
                    gpsimd.indirect_dma_start(
                        out=pool_sb[:, p % 2, cn],
                        out_offset=None,
                        in_=xT_in[:],
                        in_offset=bass.IndirectOffsetOnAxis(
                            ap=gi_sb[:, p, cn:cn + 1], axis=0
                        ),
                    ).then_inc(sem, 16)

        @block.tensor
        def _(pe: bass.BassEngine):
            for T in range(NT):
                p = T // 2
                if T >= 4:
                    pe.wait_ge(v_sem, T - 3)
                pe.wait_ge(st_sems[T % 2], 16 * (T // 2 + 1))
                if p == 0:
                    pe.wait_ge(gh0a, 16 * chalf)
                else:
                    pe.wait_ge(pair_sems[p], 16 * cmax)
                for bh in range(NBH):
                    for cn in range(cmax):
                        if p == 0 and cn == chalf and bh == 0 and T == 0:
                            pe.wait_ge(gh0b, 16 * (cmax - chalf))
                        mm = pe.matmul(
                            out=psum[T % 4][bh][:],
                            lhsT=st_sb[:, T % 2, cn * 128:(cn + 1) * 128],
                            rhs=pool_sb[:, p % 2, cn, bh * 512:(bh + 1) * 512],
                            start=(cn == 0),
                            stop=(cn == cmax - 1),
                        )
                        if bh == NBH - 1 and cn == cmax - 1:
                            mm.then_inc(pe_sem, 1)

        @block.vector
        def _(vector: bass.BassEngine):
            for T in range(NT):
                vector.wait_ge(pe_sem, T + 1)
                if T >= 2:
                    vector.wait_ge(yd_sems[T % 2], 16 * (T // 2))
                vector.tensor_copy(out=out_sb[:, T % 2, 0:512], in_=psum[T % 4][0][:])
                vector.tensor_copy(
                    out=out_sb[:, T % 2, 512:1024], in_=psum[T % 4][1][:]
                ).then_inc(v_sem, 1)

        @block.scalar
        def _(scalar: bass.BassEngine):
            for T in range(NT):
                scalar.wait_ge(v_sem, T + 1)
                scalar.dma_start(out=y_out[T], in_=out_sb[:, T % 2]).then_inc(
                    yd_sems[T % 2], 16
                )
            scalar.wait_ge(yd_sems[0], 16 * (NT // 2))
            scalar.wait_ge(yd_sems[1], 16 * (NT // 2))

    return nc


def _prep_inputs(x, connections, weights):
    xT16 = np.ascontiguousarray(x.T.astype(np.float16))        # [IN, B]

    conn = connections.reshape(NCORES, NP, 256, K)
    wts = weights.reshape(NCORES, NP, 256, K).astype(np.float32)

    uniqs = [[np.unique(conn[c, p]) for p in range(NP)] for c in range(NCORES)]
    cmax = max((len(u) + 127) // 128 for per_core in uniqs for u in per_core)

    gidx = np.zeros((NCORES, 128, NP, cmax), dtype=np.int32)
    stat = np.zeros((NCORES, NT, 128, cmax * 128), dtype=np.float16)
    for c in range(NCORES):
        for p in range(NP):
            u = uniqs[c][p]
            n_u = len(u)
            pool = np.zeros(cmax * 128, dtype=np.int64)
            pool[:n_u] = u
            # slot (cn, s): idx[s, p, cn] = pool[cn*128 + s]
            gidx[c, :, p, :] = pool.reshape(cmax, 128).T
            slots = np.searchsorted(u, conn[c, p])                 # [256, K]
            st = np.zeros((2, cmax * 128, 128), dtype=np.float32)  # [tt, slot, m]
            tt = np.repeat(np.arange(256) // 128, K).reshape(256, K)
            m = np.repeat(np.arange(256) % 128, K).reshape(256, K)
            np.add.at(st, (tt, slots, m), wts[c, p])
            for ti in range(2):
                stat[c, 2 * p + ti] = (
                    st[ti].astype(np.float16)
                    .reshape(cmax, 128, 128)       # [cn, s, m]
                    .transpose(1, 0, 2)            # [s, cn, m]
                    .reshape(128, cmax * 128)
                )
    return xT16, stat, gidx, cmax


def kernel(x, connections, weights):
    x = np.asarray(x)
    connections = np.asarray(connections)
    weights = np.asarray(weights)
    xT16, stat, gidx, cmax = _prep_inputs(x, connections, weights)
    if cmax not in _cached:
        _cached[cmax] = _build_program(cmax)
    nc = _cached[cmax]
    in_maps = [
        {"xT16": xT16, "stat": stat[c], "gidx": gidx[c]} for c in range(NCORES)
    ]
    res = run_bass_kernel_spmd(nc, in_maps, core_ids=list(range(NCORES)))
    out = np.empty((B, O), dtype=np.float32)
    for c in range(NCORES):
        y = res.results[c]["y"]
        out[:, c * OC:(c + 1) * OC] = y.reshape(OC, B).T
    return out
